# revision 29
# baseline (speedup 1.0000x reference)
# Bahdanau-attention kernel for TRN2, data-parallel over batch across 8 NeuronCores.
#
# reference math (B=16, S=2048, H=1024):
#   h_proj = hidden @ W[:, :H].T                      [B, H]
#   e_proj = einsum('bsh,gh->bsg', enc, W[:, H:])     [B, S, H]
#   scores = tanh(h_proj[:,None,:] + e_proj + b)      [B, S, H]
#   logits = scores @ v                               [B, S]
#   out    = softmax(logits, -1)[:, None, :]          [B, 1, S]
#
# Per-core (2 batches): the e_proj GEMM is fp8 DoubleRow at the PE roofline
# (~216ns per [128x512] MM covering a 256-row contraction). Phase structure:
# one phase per (batch, s-block of 512); per phase 8 g-tiles x 4 k-pair MMs
# into single-bank PSUM slots (ring of 6), drained by one tanh each
# (fp8 scores out, per-partition bias carries h_proj + b). h_proj itself is
# fp8 DR (32 tiny MMs) interleaved into the initial DMA window. v-dots are
# deferred per batch and column-packed 4-wide via tile_position into one
# PSUM bank; softmax runs on a [4, 512] partition-distributed layout
# (strided-partition exp + tiny DMA partition gathers).
import numpy as np
import ml_dtypes

import concourse.bass as bass
import concourse.mybir as mybir
import concourse.tile as tile
from concourse import bacc
from concourse.bass_utils import run_bass_kernel_spmd
from concourse.tile_rust import add_dep_helper

B, S, H = 16, 2048, 1024
NCORES = 8
BPC = B // NCORES          # batches per core
KT = H // 128              # contraction tiles
GT = H // 128              # output (g) tiles
SBLK = 512
NSB = S // SBLK

BF16 = mybir.dt.bfloat16
F32 = mybir.dt.float32
FP8 = mybir.dt.float8e4
DR = mybir.MatmulPerfMode.DoubleRow
WSCALE = 32.0

STRIDED_EXP = False        # BIR verifier rejects partition-strided ACT reads

_CACHE = {}


def _build():
    nc = bacc.Bacc("TRN2", target_bir_lowering=False, debug=False, num_devices=NCORES)

    encT_d = nc.dram_tensor("encT", [BPC, 128, NSB, KT, SBLK], FP8, kind="ExternalInput")
    we_d = nc.dram_tensor("we", [128, GT, KT, 128], FP8, kind="ExternalInput")
    whf_d = nc.dram_tensor("whf", [128, GT, KT, 128], FP8, kind="ExternalInput")
    hiddenT_d = nc.dram_tensor("hiddenT", [128, KT, BPC], FP8, kind="ExternalInput")
    bvec_d = nc.dram_tensor("bvec", [128, GT], F32, kind="ExternalInput")
    vvec_d = nc.dram_tensor("vvec", [128, GT, 16], FP8, kind="ExternalInput")
    out_d = nc.dram_tensor("out", [BPC, S], F32, kind="ExternalOutput")

    ACT = mybir.ActivationFunctionType

    with tile.TileContext(nc) as tc:
        with (
            tc.tile_pool(name="const", bufs=1) as constp,
            tc.tile_pool(name="wp", bufs=1) as wp,
            tc.tile_pool(name="encp", bufs=1) as encp,
            tc.tile_pool(name="scp", bufs=1) as scp,
            tc.tile_pool(name="smallp", bufs=2) as smallp,
            tc.tile_pool(name="mps", bufs=7, space="PSUM") as mps,
            tc.tile_pool(name="lps", bufs=1, space="PSUM") as lps,
        ):
            # ---- ACT table preload: dummy tanh with no data deps ----
            dmy = constp.tile([1, 1], F32, tag="dmy")
            nc.vector.memset(dmy[:], 0.0)
            dmy2 = constp.tile([1, 1], F32, tag="dmy2")
            nc.scalar.activation(dmy2[:], dmy[:], ACT.Tanh)

            # ---- DMA: sync queue carries the batch-0 critical path, in
            # need-order; gpsimd (SWDGE) carries bulk batch-1 enc + consts.
            hiddenT_sb = constp.tile([128, KT, BPC], FP8, tag="hiddenT")
            nc.sync.dma_start(out=hiddenT_sb[:], in_=hiddenT_d[:])
            whf_sb = wp.tile([128, GT, KT, 128], FP8, tag="whf")
            we_sb = wp.tile([128, GT, KT, 128], FP8, tag="we")
            enc_sb = [
                encp.tile([128, NSB, KT, SBLK], FP8, name=f"enc{bb}", tag=f"enc{bb}")
                for bb in range(BPC)
            ]
            # main-phase data first; whf trickles behind (h_proj MMs are
            # interleaved into the stream and tanh gating absorbs the slip)
            nc.sync.dma_start(out=we_sb[:, 0:2], in_=we_d[:, 0:2])
            nc.sync.dma_start(out=enc_sb[0][:, 0], in_=encT_d[0][:, 0])
            nc.sync.dma_start(out=whf_sb[:, 0:4], in_=whf_d[:, 0:4])
            nc.sync.dma_start(out=we_sb[:, 2:4], in_=we_d[:, 2:4])
            nc.sync.dma_start(out=enc_sb[0][:, 1], in_=encT_d[0][:, 1])
            nc.sync.dma_start(out=whf_sb[:, 4:8], in_=whf_d[:, 4:8])
            nc.sync.dma_start(out=we_sb[:, 4:8], in_=we_d[:, 4:8])
            nc.sync.dma_start(out=enc_sb[0][:, 2], in_=encT_d[0][:, 2])
            nc.sync.dma_start(out=enc_sb[0][:, 3], in_=encT_d[0][:, 3])

            b_sb = constp.tile([128, GT], F32, tag="bvec")
            nc.gpsimd.dma_start(out=b_sb[:], in_=bvec_d[:])
            v_sb = constp.tile([128, GT, 16], FP8, tag="vvec")
            nc.gpsimd.dma_start(out=v_sb[:], in_=vvec_d[:])
            # batch-1 enc: gated below on compute milestones so the batch-0
            # critical DMAs get full HBM bandwidth first.
            encb1_dmas = [
                nc.gpsimd.dma_start(out=enc_sb[1][:, 0:2], in_=encT_d[1][:, 0:2]),
                nc.gpsimd.dma_start(out=enc_sb[1][:, 2:4], in_=encT_d[1][:, 2:4]),
            ]

            # ---- h_proj (fp8 DR, g on partitions) + bias ----
            # hp lives in the lp tag: its readers (DVE bias ops) finish in
            # phase 0, long before lp_b1 cycles back into its slot.
            hp = lps.tile([128, GT, BPC], F32, tag="lp", name="hp")
            hb_sb = constp.tile([128, GT, BPC], F32, tag="hb")

            def hproj(j):
                for kp in range(KT // 2):
                    nc.tensor.matmul(
                        hp[:, j, :],
                        whf_sb[:, j, 2 * kp : 2 * kp + 2, :],
                        hiddenT_sb[:, 2 * kp : 2 * kp + 2, :],
                        start=(kp == 0),
                        stop=(kp == KT // 2 - 1),
                        perf_mode=DR,
                    )
                nc.vector.tensor_scalar(
                    hb_sb[:, j, :], hp[:, j, :],
                    1.0 / WSCALE, b_sb[:, j : j + 1],
                    mybir.AluOpType.mult, mybir.AluOpType.add,
                )

            # scores, fp8, [p, sb, j, s']
            sc_sb = [
                scp.tile([128, NSB, GT, SBLK], FP8, name=f"sc{bb}", tag=f"sc{bb}")
                for bb in range(BPC)
            ]
            # softmax epilogue state, all on partition 0 (engine APs must be
            # 32-aligned in partition base, so spreading over partitions 0..3
            # is not expressible)
            exps_row = [
                smallp.tile([1, NSB, SBLK], F32, name=f"exps{bb}", tag=f"exps{bb}")
                for bb in range(BPC)
            ]
            parts_row = [
                smallp.tile([1, NSB], F32, name=f"parts{bb}", tag=f"parts{bb}")
                for bb in range(BPC)
            ]
            rsum1 = [
                smallp.tile([1, 1], F32, name=f"rsum{bb}", tag=f"rsum{bb}")
                for bb in range(BPC)
            ]
            outrow = [
                smallp.tile([1, NSB, SBLK], F32, name=f"outrow{bb}", tag=f"outrow{bb}")
                for bb in range(BPC)
            ]

            tanh_insts = {}

            def main_phase(h, sb, interleave_hproj=False):
                for j in range(GT):
                    mp = mps.tile([128, SBLK], F32, tag="mp", name=f"mp{h}{sb}{j}")
                    for kp in range(KT // 2):
                        nc.tensor.matmul(
                            mp[:],
                            we_sb[:, j, 2 * kp : 2 * kp + 2, :],
                            enc_sb[h][:, sb, 2 * kp : 2 * kp + 2, :],
                            start=(kp == 0),
                            stop=(kp == KT // 2 - 1),
                            perf_mode=DR,
                        )
                    if interleave_hproj:
                        hproj(j)
                    tanh_insts[(h, sb, j)] = nc.scalar.activation(
                        sc_sb[h][:, sb, j, :], mp[:], ACT.Tanh,
                        bias=hb_sb[:, j, h : h + 1],
                        scale=1.0 / WSCALE,
                    )

            def vdot_batch(h):
                # plain-fp8 v-dot, column-packed: the 4 s-blocks' MMs target
                # col-groups {0,32,64,96} of one PSUM bank and execute
                # concurrently (~4ns apart); logits land on rows {0,32,64,96}
                lp = lps.tile([128, SBLK], F32, tag="lp", name=f"lp{h}")
                for j in range(GT):
                    for sb in range(NSB):
                        nc.tensor.matmul(
                            lp[32 * sb : 32 * sb + 16, :],
                            v_sb[:, j, :],
                            sc_sb[h][:, sb, j, :],
                            start=(j == 0),
                            stop=(j == GT - 1),
                            tile_position=(0, 32 * sb),
                        )
                for sb in range(NSB):
                    nc.scalar.activation(
                        exps_row[h][:, sb, :],
                        lp[32 * sb : 32 * sb + 1, :],
                        ACT.Exp,
                        scale=1.0 / 16.0,
                    )
                    # partial sums on DVE, pipelined behind the ACT exps
                    # (accum_out lowers to an extra serial scalar-queue store)
                    nc.vector.tensor_reduce(
                        parts_row[h][:, sb : sb + 1], exps_row[h][:, sb, :],
                        axis=mybir.AxisListType.X, op=mybir.AluOpType.add,
                    )
                ssum = smallp.tile([1, 1], F32, tag=f"ssum{h}", name=f"ssum{h}")
                nc.vector.tensor_reduce(
                    ssum[:], parts_row[h][:], axis=mybir.AxisListType.X,
                    op=mybir.AluOpType.add,
                )
                nc.vector.reciprocal(rsum1[h][:], ssum[:])
                nc.vector.tensor_scalar_mul(
                    outrow[h][:], exps_row[h][:], rsum1[h][:, 0:1]
                )
                nc.sync.dma_start(out=out_d[h : h + 1, :], in_=outrow[h][:])

            # ---- phases ----
            for sb in range(NSB):
                main_phase(0, sb, interleave_hproj=(sb == 0))
            vdot_batch(0)
            for sb in range(NSB):
                main_phase(1, sb)
            vdot_batch(1)

            # gate batch-1 enc DMAs on batch-0 compute progress so the
            # critical batch-0 stream gets full HBM bandwidth first
            add_dep_helper(
                encb1_dmas[0].ins, tanh_insts[(0, 0, 0)].ins, sync=True,
                reason="enc b1 first half after phase(0,0) starts draining",
            )
            add_dep_helper(
                encb1_dmas[1].ins, tanh_insts[(0, 1, 0)].ins, sync=True,
                reason="enc b1 second half after phase(0,1) starts draining",
            )

    nc.compile()
    return nc


def _get_nc():
    if "nc" not in _CACHE:
        _CACHE["nc"] = _build()
    return _CACHE["nc"]


def _make_in_maps(hidden, encoder_outputs, W, b, v):
    fp8 = ml_dtypes.float8_e4m3
    WT = np.ascontiguousarray(W.T)  # [2H, H]; WT[hin, gout]
    w_tiles = WT.reshape(2, KT, 128, GT, 128).transpose(0, 2, 3, 1, 4)  # [half, p, j, k, m]
    whf_host = np.ascontiguousarray(w_tiles[0] * WSCALE).astype(fp8)
    we_host = np.ascontiguousarray(w_tiles[1] * WSCALE).astype(fp8)
    b_host = np.ascontiguousarray(b.reshape(GT, 128).T).astype(np.float32)
    v_host = np.zeros((128, GT, 16), dtype=fp8)
    v_host[:, :, 0] = (v.reshape(GT, 128).T * 16.0).astype(fp8)

    in_maps = []
    for i in range(NCORES):
        hs = hidden[BPC * i : BPC * (i + 1)]  # [BPC, H]
        es = encoder_outputs[BPC * i : BPC * (i + 1)]  # [BPC, S, H]
        hT = np.ascontiguousarray(
            hs.T.reshape(KT, 128, BPC).transpose(1, 0, 2)
        ).astype(fp8)
        # encT[bb, p, sb, k, s'] = enc[bb, sb*512+s', 128k+p]
        eT = np.ascontiguousarray(
            es.reshape(BPC, NSB, SBLK, KT, 128).transpose(0, 4, 1, 3, 2)
        ).astype(fp8)
        in_maps.append(
            {
                "encT": eT,
                "we": we_host,
                "whf": whf_host,
                "hiddenT": hT,
                "bvec": b_host,
                "vvec": v_host,
            }
        )
    return in_maps


def _run(in_maps, **kwargs):
    nc = _get_nc()
    try:
        return run_bass_kernel_spmd(
            nc, in_maps, core_ids=list(range(NCORES)), **kwargs
        )
    except Exception:
        import time as _time

        _time.sleep(20)
        return run_bass_kernel_spmd(
            nc, in_maps, core_ids=list(range(NCORES)), **kwargs
        )


def kernel(hidden, encoder_outputs, W, b, v):
    hidden = np.asarray(hidden, dtype=np.float32)
    encoder_outputs = np.asarray(encoder_outputs, dtype=np.float32)
    W = np.asarray(W, dtype=np.float32)
    b = np.asarray(b, dtype=np.float32)
    v = np.asarray(v, dtype=np.float32)

    in_maps = _make_in_maps(hidden, encoder_outputs, W, b, v)
    res = _run(in_maps)
    outs = [np.asarray(res.results[i]["out"], dtype=np.float32) for i in range(NCORES)]
    return np.concatenate(outs, axis=0).reshape(B, 1, S)


# revision 33
# speedup vs baseline: 1.1911x; 1.1911x over previous
# Bahdanau-attention kernel for TRN2, data-parallel over batch across 8 NeuronCores.
#
# reference math (B=16, S=2048, H=1024):
#   h_proj = hidden @ W[:, :H].T                      [B, H]
#   e_proj = einsum('bsh,gh->bsg', enc, W[:, H:])     [B, S, H]
#   scores = tanh(h_proj[:,None,:] + e_proj + b)      [B, S, H]
#   logits = scores @ v                               [B, S]
#   out    = softmax(logits, -1)[:, None, :]          [B, 1, S]
#
# Per-core (2 batches): the e_proj GEMM is fp8 DoubleRow at the PE roofline
# (~216ns per [128x512] MM covering a 256-row contraction). Phase structure:
# one phase per (batch, s-block of 512); per phase 8 g-tiles x 4 k-pair MMs
# into single-bank PSUM slots (ring of 6), drained by one tanh each
# (fp8 scores out, per-partition bias carries h_proj + b). h_proj itself is
# fp8 DR (32 tiny MMs) interleaved into the initial DMA window. v-dots are
# deferred per batch and column-packed 4-wide via tile_position into one
# PSUM bank; softmax runs on a [4, 512] partition-distributed layout
# (strided-partition exp + tiny DMA partition gathers).
import numpy as np
import ml_dtypes

import concourse.bass as bass
import concourse.mybir as mybir
import concourse.tile as tile
from concourse import bacc
from concourse.bass_utils import run_bass_kernel_spmd
from concourse.tile_rust import add_dep_helper

B, S, H = 16, 2048, 1024
NCORES = 8
BPC = B // NCORES          # batches per core
KT = H // 128              # contraction tiles
GT = H // 128              # output (g) tiles
SBLK = 512
NSB = S // SBLK

BF16 = mybir.dt.bfloat16
F32 = mybir.dt.float32
FP8 = mybir.dt.float8e4
DR = mybir.MatmulPerfMode.DoubleRow
WSCALE = 32.0

STRIDED_EXP = False        # BIR verifier rejects partition-strided ACT reads

_CACHE = {}


def _build():
    nc = bacc.Bacc("TRN2", target_bir_lowering=False, debug=False, num_devices=NCORES)

    encT_d = nc.dram_tensor("encT", [BPC, 128, NSB, KT, SBLK], FP8, kind="ExternalInput")
    we_d = nc.dram_tensor("we", [128, GT, KT, 128], FP8, kind="ExternalInput")
    whf_d = nc.dram_tensor("whf", [128, GT, KT, 128], FP8, kind="ExternalInput")
    hiddenT_d = nc.dram_tensor("hiddenT", [128, KT, BPC], FP8, kind="ExternalInput")
    bvec_d = nc.dram_tensor("bvec", [128, GT], F32, kind="ExternalInput")
    vvec_d = nc.dram_tensor("vvec", [128, GT, 16], FP8, kind="ExternalInput")
    out_d = nc.dram_tensor("out", [BPC, S], F32, kind="ExternalOutput")

    ACT = mybir.ActivationFunctionType

    with tile.TileContext(nc) as tc:
        with (
            tc.tile_pool(name="const", bufs=1) as constp,
            tc.tile_pool(name="wp", bufs=1) as wp,
            tc.tile_pool(name="encp", bufs=1) as encp,
            tc.tile_pool(name="scp", bufs=1) as scp,
            tc.tile_pool(name="smallp", bufs=2) as smallp,
            tc.tile_pool(name="mps", bufs=7, space="PSUM") as mps,
            tc.tile_pool(name="lps", bufs=1, space="PSUM") as lps,
        ):
            # ---- ACT table preload: dummy tanh with no data deps ----
            dmy = constp.tile([1, 1], F32, tag="dmy")
            nc.vector.memset(dmy[:], 0.0)
            dmy2 = constp.tile([1, 1], F32, tag="dmy2")
            nc.scalar.activation(dmy2[:], dmy[:], ACT.Tanh)

            # ---- DMA: sync queue carries the batch-0 critical path, in
            # need-order; gpsimd (SWDGE) carries bulk batch-1 enc + consts.
            hiddenT_sb = constp.tile([128, KT, BPC], FP8, tag="hiddenT")
            nc.gpsimd.dma_start(out=hiddenT_sb[:], in_=hiddenT_d[:])
            whf_sb = wp.tile([128, GT, KT, 128], FP8, tag="whf")
            we_sb = wp.tile([128, GT, KT, 128], FP8, tag="we")
            enc_sb = [
                encp.tile([128, NSB, KT, SBLK], FP8, name=f"enc{bb}", tag=f"enc{bb}")
                for bb in range(BPC)
            ]
            # queue split: sync streams batch-0 enc; scalar (after its ACT
            # table preload) streams we; gpsimd (SWDGE) streams h_proj
            # operands, small consts, and gated batch-1 enc.
            nc.sync.dma_start(out=enc_sb[0][:, 0], in_=encT_d[0][:, 0])
            nc.sync.dma_start(out=enc_sb[0][:, 1], in_=encT_d[0][:, 1])
            nc.sync.dma_start(out=enc_sb[0][:, 2], in_=encT_d[0][:, 2])
            nc.sync.dma_start(out=enc_sb[0][:, 3], in_=encT_d[0][:, 3])

            nc.scalar.dma_start(out=we_sb[:, 0:2], in_=we_d[:, 0:2])
            nc.scalar.dma_start(out=we_sb[:, 2:4], in_=we_d[:, 2:4])
            nc.scalar.dma_start(out=we_sb[:, 4:8], in_=we_d[:, 4:8])

            nc.gpsimd.dma_start(out=whf_sb[:, 0:4], in_=whf_d[:, 0:4])
            nc.gpsimd.dma_start(out=whf_sb[:, 4:8], in_=whf_d[:, 4:8])
            b_sb = constp.tile([128, GT], F32, tag="bvec")
            nc.gpsimd.dma_start(out=b_sb[:], in_=bvec_d[:])
            v_sb = constp.tile([128, GT, 16], FP8, tag="vvec")
            nc.gpsimd.dma_start(out=v_sb[:], in_=vvec_d[:])
            # batch-1 enc: gated below on compute milestones so the batch-0
            # critical DMAs get full HBM bandwidth first.
            encb1_dmas = [
                nc.gpsimd.dma_start(out=enc_sb[1][:, 0:2], in_=encT_d[1][:, 0:2]),
                nc.gpsimd.dma_start(out=enc_sb[1][:, 2:4], in_=encT_d[1][:, 2:4]),
            ]

            # ---- h_proj (fp8 DR, g on partitions) + bias ----
            # hp lives in the lp tag: its readers (DVE bias ops) finish in
            # phase 0, long before lp_b1 cycles back into its slot.
            hp = lps.tile([128, GT, BPC], F32, tag="lp", name="hp")
            hb_sb = constp.tile([128, GT, BPC], F32, tag="hb")

            def hproj(j):
                # plain fp8 (no DoubleRow): FWL makes the weight loads ~27ns,
                # and at N=2 the MMs are issue-floor bound (~81ns each)
                for k in range(KT):
                    nc.tensor.matmul(
                        hp[:, j, :],
                        whf_sb[:, j, k, :],
                        hiddenT_sb[:, k, :],
                        start=(k == 0),
                        stop=(k == KT - 1),
                    )
                nc.vector.tensor_scalar(
                    hb_sb[:, j, :], hp[:, j, :],
                    1.0 / WSCALE, b_sb[:, j : j + 1],
                    mybir.AluOpType.mult, mybir.AluOpType.add,
                )

            # scores, fp8, [p, sb, j, s']
            sc_sb = [
                scp.tile([128, NSB, GT, SBLK], FP8, name=f"sc{bb}", tag=f"sc{bb}")
                for bb in range(BPC)
            ]
            # softmax epilogue state, all on partition 0 (engine APs must be
            # 32-aligned in partition base, so spreading over partitions 0..3
            # is not expressible)
            exps_row = [
                smallp.tile([1, NSB, SBLK], F32, name=f"exps{bb}", tag=f"exps{bb}")
                for bb in range(BPC)
            ]
            parts_row = [
                smallp.tile([1, NSB], F32, name=f"parts{bb}", tag=f"parts{bb}")
                for bb in range(BPC)
            ]
            rsum1 = [
                smallp.tile([1, 1], F32, name=f"rsum{bb}", tag=f"rsum{bb}")
                for bb in range(BPC)
            ]
            outrow = [
                smallp.tile([1, NSB, SBLK], F32, name=f"outrow{bb}", tag=f"outrow{bb}")
                for bb in range(BPC)
            ]

            tanh_insts = {}

            def main_phase(h, sb, interleave_hproj=False):
                for j in range(GT):
                    mp = mps.tile([128, SBLK], F32, tag="mp", name=f"mp{h}{sb}{j}")
                    for kp in range(KT // 2):
                        nc.tensor.matmul(
                            mp[:],
                            we_sb[:, j, 2 * kp : 2 * kp + 2, :],
                            enc_sb[h][:, sb, 2 * kp : 2 * kp + 2, :],
                            start=(kp == 0),
                            stop=(kp == KT // 2 - 1),
                            perf_mode=DR,
                        )
                    if interleave_hproj:
                        hproj(j)
                    tanh_insts[(h, sb, j)] = nc.scalar.activation(
                        sc_sb[h][:, sb, j, :], mp[:], ACT.Tanh,
                        bias=hb_sb[:, j, h : h + 1],
                        scale=1.0 / WSCALE,
                    )

            def vdot_batch(h):
                # plain-fp8 v-dot, column-packed: the 4 s-blocks' MMs target
                # col-groups {0,32,64,96} of one PSUM bank and execute
                # concurrently (~4ns apart); logits land on rows {0,32,64,96}
                lp = lps.tile([128, SBLK], F32, tag="lp", name=f"lp{h}")
                for j in range(GT):
                    for sb in range(NSB):
                        nc.tensor.matmul(
                            lp[32 * sb : 32 * sb + 16, :],
                            v_sb[:, j, :],
                            sc_sb[h][:, sb, j, :],
                            start=(j == 0),
                            stop=(j == GT - 1),
                            tile_position=(0, 32 * sb),
                        )
                for sb in range(NSB):
                    nc.scalar.activation(
                        exps_row[h][:, sb, :],
                        lp[32 * sb : 32 * sb + 1, :],
                        ACT.Exp,
                        scale=1.0 / 16.0,
                    )
                    # partial sums on DVE, pipelined behind the ACT exps
                    # (accum_out lowers to an extra serial scalar-queue store)
                    nc.vector.tensor_reduce(
                        parts_row[h][:, sb : sb + 1], exps_row[h][:, sb, :],
                        axis=mybir.AxisListType.X, op=mybir.AluOpType.add,
                    )
                ssum = smallp.tile([1, 1], F32, tag=f"ssum{h}", name=f"ssum{h}")
                nc.vector.tensor_reduce(
                    ssum[:], parts_row[h][:], axis=mybir.AxisListType.X,
                    op=mybir.AluOpType.add,
                )
                nc.vector.reciprocal(rsum1[h][:], ssum[:])
                # normalize split across DVE (3 chunks) and ACT (1 chunk)
                nc.vector.tensor_scalar_mul(
                    outrow[h][:, 0:3, :], exps_row[h][:, 0:3, :],
                    rsum1[h][:, 0:1],
                )
                nc.scalar.activation(
                    outrow[h][:, 3, :], exps_row[h][:, 3, :], ACT.Copy,
                    scale=rsum1[h][:, 0:1],
                )
                nc.sync.dma_start(out=out_d[h : h + 1, :], in_=outrow[h][:])

            # ---- phases ----
            for sb in range(NSB):
                main_phase(0, sb, interleave_hproj=(sb == 0))
            vdot_batch(0)
            for sb in range(NSB):
                main_phase(1, sb)
            vdot_batch(1)

            # gate batch-1 enc DMAs on batch-0 compute progress so the
            # critical batch-0 stream gets full HBM bandwidth first
            add_dep_helper(
                encb1_dmas[0].ins, tanh_insts[(0, 0, 0)].ins, sync=True,
                reason="enc b1 first half after phase(0,0) starts draining",
            )
            add_dep_helper(
                encb1_dmas[1].ins, tanh_insts[(0, 1, 0)].ins, sync=True,
                reason="enc b1 second half after phase(0,1) starts draining",
            )

    nc.compile()
    return nc


def _get_nc():
    if "nc" not in _CACHE:
        _CACHE["nc"] = _build()
    return _CACHE["nc"]


def _make_in_maps(hidden, encoder_outputs, W, b, v):
    fp8 = ml_dtypes.float8_e4m3
    WT = np.ascontiguousarray(W.T)  # [2H, H]; WT[hin, gout]
    w_tiles = WT.reshape(2, KT, 128, GT, 128).transpose(0, 2, 3, 1, 4)  # [half, p, j, k, m]
    whf_host = np.ascontiguousarray(w_tiles[0] * WSCALE).astype(fp8)
    we_host = np.ascontiguousarray(w_tiles[1] * WSCALE).astype(fp8)
    b_host = np.ascontiguousarray(b.reshape(GT, 128).T).astype(np.float32)
    v_host = np.zeros((128, GT, 16), dtype=fp8)
    v_host[:, :, 0] = (v.reshape(GT, 128).T * 16.0).astype(fp8)

    in_maps = []
    for i in range(NCORES):
        hs = hidden[BPC * i : BPC * (i + 1)]  # [BPC, H]
        es = encoder_outputs[BPC * i : BPC * (i + 1)]  # [BPC, S, H]
        hT = np.ascontiguousarray(
            hs.T.reshape(KT, 128, BPC).transpose(1, 0, 2)
        ).astype(fp8)
        # encT[bb, p, sb, k, s'] = enc[bb, sb*512+s', 128k+p]
        eT = np.ascontiguousarray(
            es.reshape(BPC, NSB, SBLK, KT, 128).transpose(0, 4, 1, 3, 2)
        ).astype(fp8)
        in_maps.append(
            {
                "encT": eT,
                "we": we_host,
                "whf": whf_host,
                "hiddenT": hT,
                "bvec": b_host,
                "vvec": v_host,
            }
        )
    return in_maps


def _run(in_maps, **kwargs):
    nc = _get_nc()
    try:
        return run_bass_kernel_spmd(
            nc, in_maps, core_ids=list(range(NCORES)), **kwargs
        )
    except Exception:
        import time as _time

        _time.sleep(20)
        return run_bass_kernel_spmd(
            nc, in_maps, core_ids=list(range(NCORES)), **kwargs
        )


def kernel(hidden, encoder_outputs, W, b, v):
    hidden = np.asarray(hidden, dtype=np.float32)
    encoder_outputs = np.asarray(encoder_outputs, dtype=np.float32)
    W = np.asarray(W, dtype=np.float32)
    b = np.asarray(b, dtype=np.float32)
    v = np.asarray(v, dtype=np.float32)

    in_maps = _make_in_maps(hidden, encoder_outputs, W, b, v)
    res = _run(in_maps)
    outs = [np.asarray(res.results[i]["out"], dtype=np.float32) for i in range(NCORES)]
    return np.concatenate(outs, axis=0).reshape(B, 1, S)
